# revision 16
# baseline (speedup 1.0000x reference)
"""Trainium2 Bass kernel for nn_DiffusionTransformerBlock (AF3-style block).

Self-contained: hardcodes shapes B=1, N=1024, c_a=768, c_s=384, c_z=128, H=16.
Shards the 1024 query tokens across 8 NeuronCores (128 per core); x/k/v are
computed replicated on every core (no collectives). The z pair-bias branch is
query-sharded with the scores. Matmuls run in bf16 (fp32 is 4x slower on PE);
LN statistics, softmax accumulation and the residual path stay in fp32.

Host-side folds:
  - s_ln affine (g,b) folded into the four adaLN weight matrices + bias vecs
  - q scale D^-0.5 folded into Wq/bq; head dim padded 48->64 for 32-aligned
    partition offsets
  - z-LN affine folded into Wz; per-position mean term folded in as
    Wz'' = g*Wz - ones*colsum(g*Wz)/128; the per-head constant (zln_b @ Wz)
    is softmax-invariant and dropped. Mean and E[z^2] come from extra matmul
    columns; inv-std is applied elementwise on the DVE.
"""
import sys
sys.path.insert(0, "/opt/trn_rl_repo")
import numpy as np
import ml_dtypes

from concourse import bacc, tile, mybir, bass_utils

bf16 = ml_dtypes.bfloat16
F32 = mybir.dt.float32
BF16 = mybir.dt.bfloat16
AF = mybir.ActivationFunctionType
MULT = mybir.AluOpType.mult
ADD = mybir.AluOpType.add
SUB = mybir.AluOpType.subtract

NCORES = 8
P = 128
N = 1024
CA = 768
CS = 384
CZ = 128
H = 16
D = 48
HP = H // 2        # 8 head-pair fo-tiles (head dim padded 48->64)
NQ = N // NCORES   # 128 own query tokens
CAT = CA // P      # 6
CST = CS // P      # 3
NT = N // P        # 8 token tiles
F1 = 2 * CA        # 1536
F1T = F1 // P      # 12
EPS = 1e-5
ZCH = 4            # q-rows per z chunk
NCH = NQ // ZCH    # 32 chunks
KT = N // P        # 8 k-tiles

_CACHE = {}


def build_kernel():
    nc = bacc.Bacc("TRN2", target_bir_lowering=False, debug=False,
                   enable_asserts=True, num_devices=NCORES)

    dr = lambda name, shape, dt: nc.dram_tensor(name, shape, dt, kind="ExternalInput").ap()
    a_full = dr("a_full", [N, CA], F32)
    s_full = dr("s_full", [N, CS], F32)
    a_own = dr("a_own", [NQ, CA], F32)
    s_own = dr("s_own", [NQ, CS], F32)
    z_sh = dr("z_sh", [NQ * N, CZ], BF16)
    w_sg1 = dr("w_sg1", [CS, CA], BF16)
    w_sb1 = dr("w_sb1", [CS, CA], BF16)
    w_q = dr("w_q", [CA, HP * P], BF16)
    w_k = dr("w_k", [CA, HP * P], BF16)
    w_v = dr("w_v", [CA, CA], BF16)
    w_g = dr("w_g", [CA, CA], BF16)
    w_o = dr("w_o", [CA, CA], BF16)
    w_sg2 = dr("w_sg2", [CS, CA], BF16)
    w_sb2 = dr("w_sb2", [CS, CA], BF16)
    w_1 = dr("w_1", [CA, F1], BF16)
    w_2 = dr("w_2", [CA, F1], BF16)
    w_b = dr("w_b", [F1, CA], BF16)
    w_s = dr("w_s", [CS, CA], BF16)
    w_last = dr("w_last", [CS, CA], BF16)
    w_z = dr("w_z", [CZ, 18], BF16)            # [Wz''(16) | 1/128 | 1/128]
    bias_blob = dr("bias_blob", [P, 20], F32)    # bsg1(0:6) bsb1(6:12) bq(12:20)
    bias_blob2 = dr("bias_blob2", [P, 12], F32)  # bg2(0:6) bb2(6:12)
    bias_rows = dr("bias_rows", [1, 3 * CA], BF16)  # blast | bo | bs
    ident_in = dr("ident_in", [P, P], BF16)
    ones_row_in = dr("ones_row_in", [1, P], BF16)
    ones_col_in = dr("ones_col_in", [P, 1], BF16)

    out_own = nc.dram_tensor("out_own", [NQ, CA], F32, kind="ExternalOutput").ap()
    taps = {}
    if _CACHE.get("debug_taps"):
        for nm, shape, dt in [
            ("t_otok", [P, CA], BF16), ("t_r", [P, H], F32),
            ("t_attn", [P, CA], F32), ("t_gate2", [P, CA], BF16),
            ("t_bias", [P, KT * H * NQ], BF16), ("t_qpad", [P, HP * NQ], BF16),
            ("t_kpad", [P, HP * N], BF16), ("t_vtok", [P, NT * CA], BF16),
            ("t_xown", [P, CAT * NQ], BF16), ("t_gateown", [P, CA], BF16),
        ]:
            taps[nm] = nc.dram_tensor(nm, shape, dt, kind="ExternalOutput").ap()

    z3 = z_sh.rearrange("(q k) c -> q k c", k=N)

    from contextlib import ExitStack
    with tile.TileContext(nc) as tc, ExitStack() as es:
        # ---- long-lived pools ----------------------------------------
        cpool = es.enter_context(tc.tile_pool(name="cpool", bufs=1))
        own = es.enter_context(tc.tile_pool(name="own", bufs=1))
        persist = es.enter_context(tc.tile_pool(name="persist", bufs=1))
        psA = es.enter_context(tc.tile_pool(name="psA", bufs=2, space="PSUM"))
        psB = es.enter_context(tc.tile_pool(name="psB", bufs=4, space="PSUM"))
        zes = ExitStack()
        zpool = zes.enter_context(tc.tile_pool(name="zpool", bufs=1))

        def load_w(pool, ap_dram, cdim, fdim, name):
            t = pool.tile([P, cdim // P, fdim], BF16, name=name)
            nc.sync.dma_start(t[:], ap_dram.rearrange("(t p) f -> p t f", p=P))
            return t

        # constants
        Wz = cpool.tile([P, 18], BF16, name="Wz")
        nc.sync.dma_start(Wz[:], w_z[:])
        bb1 = cpool.tile([P, 20], F32, name="bb1")
        nc.sync.dma_start(bb1[:], bias_blob[:])
        bb2 = cpool.tile([P, 12], F32, name="bb2")
        nc.sync.dma_start(bb2[:], bias_blob2[:])
        brows = cpool.tile([1, 3 * CA], BF16, name="brows")
        nc.sync.dma_start(brows[:], bias_rows[:])
        ident = cpool.tile([P, P], BF16, name="ident")
        nc.sync.dma_start(ident[:], ident_in[:])
        ones_row = cpool.tile([1, P], BF16, name="ones_row")
        nc.sync.dma_start(ones_row[:], ones_row_in[:])
        ones_col = cpool.tile([P, 1], BF16, name="ones_col")
        nc.sync.dma_start(ones_col[:], ones_col_in[:])
        eps_t = cpool.tile([P, 1], F32, name="eps_t")
        nc.vector.memset(eps_t[:], EPS)
        lnsq_scratch = cpool.tile([P, CA], F32, name="lnsq_scratch")

        # persistent activations
        bias_all = persist.tile([P, KT, H, NQ], BF16, name="bias_all")
        k_pad = persist.tile([P, HP, N], BF16, name="k_pad")
        v_tok = persist.tile([P, NT, CA], BF16, name="v_tok")
        q_pad = persist.tile([P, HP, NQ], BF16, name="q_pad")
        gate_own = persist.tile([P, CA], BF16, name="gate_own")
        o_tok = persist.tile([P, CA], BF16, name="o_tok")
        r_sb = persist.tile([P, H], F32, name="r_sb")
        attn_out = persist.tile([P, CA], F32, name="attn_out")
        sT_own = own.tile([P, CST, NQ], BF16, name="sT_own")
        s_own_lnT = own.tile([P, CST, NQ], BF16, name="s_own_lnT")
        x_own = own.tile([P, CAT, NQ], BF16, name="x_own")
        a_own_lnT = own.tile([P, CAT, NQ], BF16, name="a_own_lnT")

        # ---- helpers --------------------------------------------------
        def ln_stats(pool, x_ap, nfeat, tag):
            """x [128, nfeat] f32 -> (inv, negmi) [128,1] f32."""
            ssum = pool.tile([P, 1], F32, tag="ln_a", name=f"ss_{tag}")
            nc.scalar.activation(lnsq_scratch[:, 0:nfeat], x_ap, AF.Square,
                                 accum_out=ssum[:])
            negmean = pool.tile([P, 1], F32, tag="ln_b", name=f"nm_{tag}")
            nc.vector.reduce_sum(negmean[:], x_ap, axis=mybir.AxisListType.X)
            nc.vector.tensor_scalar_mul(negmean[:], negmean[:], -1.0 / nfeat)
            var = pool.tile([P, 1], F32, tag="ln_c", name=f"var_{tag}")
            nc.vector.tensor_scalar_mul(var[:], ssum[:], 1.0 / nfeat)
            msq = pool.tile([P, 1], F32, tag="ln_d", name=f"msq_{tag}")
            nc.vector.tensor_tensor(msq[:], negmean[:], negmean[:], op=MULT)
            nc.vector.tensor_tensor(var[:], var[:], msq[:], op=SUB)
            std = pool.tile([P, 1], F32, tag="ln_e", name=f"std_{tag}")
            nc.scalar.activation(std[:], var[:], AF.Sqrt, bias=eps_t[:])
            inv = pool.tile([P, 1], F32, tag="ln_f", name=f"inv_{tag}")
            nc.vector.reciprocal(inv[:], std[:])
            negmi = pool.tile([P, 1], F32, tag="ln_g", name=f"nmi_{tag}")
            nc.vector.tensor_tensor(negmi[:], negmean[:], inv[:], op=MULT)
            return inv, negmi

        def transposes(pairs, tag):
            """pairs: list of (src [128,128] bf16 AP, dst [128,128] bf16 AP)."""
            for j0 in range(0, len(pairs), 4):
                grp = pairs[j0:j0 + 4]
                tp = psB.tile([P, 4, P], BF16, tag="psB", name=f"tp_{tag}_{j0}")
                for i, (src, _) in enumerate(grp):
                    nc.tensor.transpose(tp[:, i, :], src, ident[:])
                for i, (_, dst) in enumerate(grp):
                    nc.scalar.copy(dst, tp[:, i, :])

        # ================================================================
        # Phases 1-3 (scoped pools, freed before the epilogue)
        # ================================================================
        with tc.tile_pool(name="wearly", bufs=1) as we, \
             tc.tile_pool(name="ph12", bufs=1) as p12, \
             tc.tile_pool(name="wk1", bufs=2) as wk:
            W_sg1 = load_w(we, w_sg1, CS, CA, "W_sg1")
            W_sb1 = load_w(we, w_sb1, CS, CA, "W_sb1")
            W_q = load_w(we, w_q, CA, HP * P, "W_q")
            W_k = load_w(we, w_k, CA, HP * P, "W_k")
            W_v = load_w(we, w_v, CA, CA, "W_v")
            W_g = load_w(we, w_g, CA, CA, "W_g")

            a_lnT = p12.tile([P, CAT, N], BF16, name="a_lnT")
            s_lnT = p12.tile([P, CST, N], BF16, name="s_lnT")
            x_full = p12.tile([P, CAT, N], BF16, name="x_full")

            # --- LN(a), LN(s) over all tokens + transposes
            for i in range(NT):
                a_t = wk.tile([P, CA], F32, tag="a_t", name=f"a_t{i}", bufs=1)
                nc.sync.dma_start(a_t[:], a_full[i * P:(i + 1) * P, :])
                inv, negmi = ln_stats(wk, a_t[:], CA, f"a{i}")
                a_ln = wk.tile([P, CA], BF16, tag="a_ln", name=f"a_ln{i}")
                nc.scalar.activation(a_ln[:], a_t[:], AF.Identity,
                                     bias=negmi[:], scale=inv[:])
                s_t = wk.tile([P, CS], F32, tag="s_t", name=f"s_t{i}", bufs=1)
                nc.sync.dma_start(s_t[:], s_full[i * P:(i + 1) * P, :])
                inv2, negmi2 = ln_stats(wk, s_t[:], CS, f"s{i}")
                s_ln = wk.tile([P, CS], BF16, tag="s_ln", name=f"s_ln{i}")
                nc.scalar.activation(s_ln[:], s_t[:], AF.Identity,
                                     bias=negmi2[:], scale=inv2[:])
                transposes([(a_ln[:, j * P:(j + 1) * P],
                             a_lnT[:, j, i * P:(i + 1) * P]) for j in range(CAT)],
                           f"at{i}")
                transposes([(s_ln[:, j * P:(j + 1) * P],
                             s_lnT[:, j, i * P:(i + 1) * P]) for j in range(CST)],
                           f"st{i}")

            # --- own-token LN (a_own, s_own) + raw-s transpose
            a_ot = wk.tile([P, CA], F32, tag="a_t", name="a_ot", bufs=1)
            nc.sync.dma_start(a_ot[:], a_own[:])
            invo, negmio = ln_stats(wk, a_ot[:], CA, "aown")
            a_oln = wk.tile([P, CA], BF16, tag="a_ln", name="a_oln")
            nc.scalar.activation(a_oln[:], a_ot[:], AF.Identity,
                                 bias=negmio[:], scale=invo[:])
            transposes([(a_oln[:, j * P:(j + 1) * P], a_own_lnT[:, j, :])
                        for j in range(CAT)], "aot")
            s_ot = wk.tile([P, CS], F32, tag="s_t", name="s_ot", bufs=1)
            nc.sync.dma_start(s_ot[:], s_own[:])
            invso, negmiso = ln_stats(wk, s_ot[:], CS, "sown")
            s_oln = wk.tile([P, CS], BF16, tag="s_ln", name="s_oln")
            nc.scalar.activation(s_oln[:], s_ot[:], AF.Identity,
                                 bias=negmiso[:], scale=invso[:])
            transposes([(s_oln[:, j * P:(j + 1) * P], s_own_lnT[:, j, :])
                        for j in range(CST)], "sot")
            s_oraw = wk.tile([P, CS], BF16, tag="s_ln", name="s_oraw")
            nc.scalar.copy(s_oraw[:], s_ot[:])
            transposes([(s_oraw[:, j * P:(j + 1) * P], sT_own[:, j, :])
                        for j in range(CST)], "sor")

            # --- x (feature layout) for all tokens
            for j in range(CAT):
                for b in range(2):
                    sl = slice(b * 512, (b + 1) * 512)
                    pg = psB.tile([P, 512], F32, tag="psB", name=f"pg{j}{b}")
                    pb = psB.tile([P, 512], F32, tag="psB", name=f"pb{j}{b}")
                    for i in range(CST):
                        nc.tensor.matmul(pg[:], W_sg1[:, i, j * P:(j + 1) * P],
                                         s_lnT[:, i, sl],
                                         start=(i == 0), stop=(i == CST - 1))
                    for i in range(CST):
                        nc.tensor.matmul(pb[:], W_sb1[:, i, j * P:(j + 1) * P],
                                         s_lnT[:, i, sl],
                                         start=(i == 0), stop=(i == CST - 1))
                    sig = wk.tile([P, 512], BF16, tag="xsig", name=f"sig{j}{b}", bufs=1)
                    nc.scalar.activation(sig[:], pg[:], AF.Sigmoid,
                                         bias=bb1[:, j:j + 1])
                    sbv = wk.tile([P, 512], BF16, tag="xsb", name=f"sbv{j}{b}", bufs=1)
                    nc.scalar.activation(sbv[:], pb[:], AF.Identity,
                                         bias=bb1[:, 6 + j:7 + j])
                    nc.vector.tensor_tensor(sig[:], sig[:], a_lnT[:, j, sl], op=MULT)
                    nc.vector.tensor_tensor(x_full[:, j, sl], sig[:], sbv[:], op=ADD)

            # --- x_own (feature layout)
            for j in range(CAT):
                pg = psB.tile([P, NQ], F32, tag="psB", name=f"pgo{j}")
                pb = psB.tile([P, NQ], F32, tag="psB", name=f"pbo{j}")
                for i in range(CST):
                    nc.tensor.matmul(pg[:], W_sg1[:, i, j * P:(j + 1) * P],
                                     s_own_lnT[:, i, :],
                                     start=(i == 0), stop=(i == CST - 1))
                for i in range(CST):
                    nc.tensor.matmul(pb[:], W_sb1[:, i, j * P:(j + 1) * P],
                                     s_own_lnT[:, i, :],
                                     start=(i == 0), stop=(i == CST - 1))
                sig = wk.tile([P, NQ], BF16, tag="osig", name=f"sigo{j}")
                nc.scalar.activation(sig[:], pg[:], AF.Sigmoid, bias=bb1[:, j:j + 1])
                sbv = wk.tile([P, NQ], BF16, tag="osb", name=f"sbvo{j}")
                nc.scalar.activation(sbv[:], pb[:], AF.Identity,
                                     bias=bb1[:, 6 + j:7 + j])
                nc.vector.tensor_tensor(sig[:], sig[:], a_own_lnT[:, j, :], op=MULT)
                nc.vector.tensor_tensor(x_own[:, j, :], sig[:], sbv[:], op=ADD)

            # --- k_pad [P, HP, N] (feature layout, padded heads)
            for j in range(HP):
                for b in range(2):
                    sl = slice(b * 512, (b + 1) * 512)
                    ps = psB.tile([P, 512], F32, tag="psB", name=f"kps{j}{b}")
                    for i in range(CAT):
                        nc.tensor.matmul(ps[:], W_k[:, i, j * P:(j + 1) * P],
                                         x_full[:, i, sl],
                                         start=(i == 0), stop=(i == CAT - 1))
                    nc.scalar.copy(k_pad[:, j, sl], ps[:])

            # --- q_pad [P, HP, NQ] from x_own (+ folded bias)
            for j in range(HP):
                ps = psB.tile([P, NQ], F32, tag="psB", name=f"qps{j}")
                for i in range(CAT):
                    nc.tensor.matmul(ps[:], W_q[:, i, j * P:(j + 1) * P],
                                     x_own[:, i, :],
                                     start=(i == 0), stop=(i == CAT - 1))
                nc.scalar.activation(q_pad[:, j, :], ps[:], AF.Identity,
                                     bias=bb1[:, 12 + j:13 + j])

            # --- v_tok [P, NT, CA] (token layout)
            for i in range(NT):
                for (n0, nn) in [(0, 512), (512, 256)]:
                    ps = psB.tile([P, 512], F32, tag="psB", name=f"vps{i}{n0}")
                    for c in range(CAT):
                        nc.tensor.matmul(ps[:, 0:nn], x_full[:, c, i * P:(i + 1) * P],
                                         W_v[:, c, n0:n0 + nn],
                                         start=(c == 0), stop=(c == CAT - 1))
                    nc.scalar.copy(v_tok[:, i, n0:n0 + nn], ps[:, 0:nn])

            # --- gate_own = sigmoid(x_own @ Wg) (token layout)
            for (n0, nn) in [(0, 512), (512, 256)]:
                ps = psB.tile([P, 512], F32, tag="psB", name=f"gps{n0}")
                for c in range(CAT):
                    nc.tensor.matmul(ps[:, 0:nn], x_own[:, c, :],
                                     W_g[:, c, n0:n0 + nn],
                                     start=(c == 0), stop=(c == CAT - 1))
                nc.scalar.activation(gate_own[:, n0:n0 + nn], ps[:, 0:nn], AF.Sigmoid)

        # ================================================================
        # Phase 4: z pair-bias branch -> bias_all  (zpool: disjoint region,
        # so its DMAs/compute overlap phases 1-3)
        # ================================================================
        for ci in range(NCH):
            zc = zpool.tile([P, ZCH * N], BF16, tag="zc", name=f"zc{ci}", bufs=2)
            nc.sync.dma_start(zc[:], z3[ci * ZCH:(ci + 1) * ZCH].rearrange(
                "q k c -> (q k) c"), transpose=True)
            z2c = zpool.tile([P, ZCH * N], BF16, tag="z2c", name=f"z2c{ci}", bufs=2)
            if ci % 2 == 0:
                nc.vector.tensor_tensor(z2c[:], zc[:], zc[:], op=MULT)
            else:
                nc.scalar.activation(z2c[:], zc[:], AF.Square)
            pz = psA.tile([P, ZCH, KT, 18], F32, tag="psA", name=f"pz{ci}")
            pbias = pz[:, :, :, 0:17]
            psq = pz[:, :, :, 17:18]
            for q in reversed(range(ZCH)):
                for t in reversed(range(KT)):
                    sl = slice(q * N + t * P, q * N + (t + 1) * P)
                    nc.tensor.matmul(pz[:, q, t, 0:17], zc[:, sl], Wz[:, 0:17],
                                     start=True, stop=True)
                    nc.tensor.matmul(pz[:, q, t, 17:18], z2c[:, sl], Wz[:, 17:18],
                                     start=True, stop=True)
            m_ap = pbias[:, :, :, 16]
            m_sb = zpool.tile([P, ZCH * KT], F32, tag="zm", name=f"zm{ci}", bufs=2)
            nc.scalar.copy(m_sb[:].rearrange("p (q t) -> p q t", q=ZCH), m_ap)
            msq = zpool.tile([P, ZCH * KT], F32, tag="zmsq", name=f"zmsq{ci}", bufs=2)
            m3 = msq[:].rearrange("p (q t) -> p q t", q=ZCH)
            nc.vector.tensor_tensor(msq[:], m_sb[:], m_sb[:], op=MULT)
            var = zpool.tile([P, ZCH * KT], F32, tag="zvar", name=f"zvar{ci}", bufs=2)
            v3 = var[:].rearrange("p (q t) -> p q t", q=ZCH)
            nc.vector.tensor_tensor(v3, psq[:, :, :, 0], m3, op=SUB)
            nc.scalar.activation(var[:], var[:], AF.Sqrt, bias=eps_t[:])
            inv = zpool.tile([P, ZCH * KT], F32, tag="zinv", name=f"zinv{ci}", bufs=2)
            nc.vector.reciprocal(inv[:], var[:])
            inv_b = inv[:].rearrange("p (q t) -> p q t", q=ZCH).broadcast_to(
                [P, ZCH, KT, H])
            dst = bias_all[:, :, :, ci * ZCH:(ci + 1) * ZCH].rearrange(
                "p t h q -> p q t h")
            nc.vector.tensor_tensor(dst, pbias[:, :, :, 0:16], inv_b, op=MULT)
        zes.close()

        # ================================================================
        # Phase 5-8: attention + epilogue
        # ================================================================
        with tc.tile_pool(name="wlate", bufs=1) as wl, \
             tc.tile_pool(name="wk3", bufs=2) as wk3:
            W_o = load_w(wl, w_o, CA, CA, "W_o")
            W_sg2 = load_w(wl, w_sg2, CS, CA, "W_sg2")
            W_sb2 = load_w(wl, w_sb2, CS, CA, "W_sb2")
            W_1 = load_w(wl, w_1, CA, F1, "W_1")
            W_2 = load_w(wl, w_2, CA, F1, "W_2")
            W_b = load_w(wl, w_b, F1, CA, "W_b")
            W_s = load_w(wl, w_s, CS, CA, "W_s")
            W_last = load_w(wl, w_last, CS, CA, "W_last")
            a_own_sb = wk3.tile([P, CA], F32, name="a_own_sb", tag="a_own_sb", bufs=1)
            nc.sync.dma_start(a_own_sb[:], a_own[:])

            # --- attention
            for h in range(H):
                hp, off = h // 2, 64 * (h % 2)
                ps_s = psA.tile([P, KT, NQ], F32, tag="psA", name=f"sc{h}")
                for t in range(KT):
                    nc.tensor.matmul(ps_s[:, t, :], ident[:], bias_all[:, t, h, :],
                                     start=True, stop=False)
                    nc.tensor.matmul(ps_s[:, t, :],
                                     k_pad[off:off + 64, hp, t * P:(t + 1) * P],
                                     q_pad[off:off + 64, hp, :],
                                     start=False, stop=True)
                exp_t = wk3.tile([P, KT, NQ], BF16, tag="exp", name=f"exp{h}")
                nc.scalar.activation(exp_t[:], ps_s[:], AF.Exp)
                ps_o = psB.tile([P, D + 1], F32, tag="psB", name=f"po{h}")
                for t in range(KT):
                    nc.tensor.matmul(ps_o[:, D:D + 1], exp_t[:, t, :], ones_col[:],
                                     start=(t == 0), stop=(t == KT - 1))
                for t in range(KT):
                    nc.tensor.matmul(ps_o[:, 0:D], exp_t[:, t, :],
                                     v_tok[:, t, h * D:(h + 1) * D],
                                     start=(t == 0), stop=(t == KT - 1))
                nc.vector.reciprocal(r_sb[:, h:h + 1], ps_o[:, D:D + 1])
                nc.vector.tensor_scalar(o_tok[:, h * D:(h + 1) * D], ps_o[:, 0:D],
                                        r_sb[:, h:h + 1], None, op0=MULT)

            # --- og = gate * o ; transpose
            og = wk3.tile([P, CA], BF16, tag="og", name="og", bufs=1)
            nc.vector.tensor_tensor(og[:], o_tok[:], gate_own[:], op=MULT)
            ogT = wk3.tile([P, CAT, NQ], BF16, tag="ogT", name="ogT", bufs=1)
            transposes([(og[:, j * P:(j + 1) * P], ogT[:, j, :])
                        for j in range(CAT)], "og")

            def token_mm(lhsT_list, w_sb, brow_idx, name):
                ps = psA.tile([P, CA], F32, tag="psA", name=name)
                nct = len(lhsT_list)
                for (n0, nn) in [(0, 512), (512, 256)]:
                    for i in range(nct):
                        nc.tensor.matmul(ps[:, n0:n0 + nn], lhsT_list[i],
                                         w_sb[:, i, n0:n0 + nn],
                                         start=(i == 0),
                                         stop=(i == nct - 1 and brow_idx is None))
                    if brow_idx is not None:
                        nc.tensor.matmul(ps[:, n0:n0 + nn], ones_row[:],
                                         brows[:, brow_idx * CA + n0:
                                               brow_idx * CA + n0 + nn],
                                         start=False, stop=True)
                return ps

            ps_ao = token_mm([ogT[:, j, :] for j in range(CAT)], W_o, 1, "ps_ao")
            ps_g2 = token_mm([sT_own[:, j, :] for j in range(CST)], W_last, 0, "ps_g2")
            gate2 = wk3.tile([P, CA], BF16, tag="gate2", name="gate2", bufs=1)
            nc.scalar.activation(gate2[:], ps_g2[:], AF.Sigmoid)
            nc.vector.tensor_tensor(attn_out[:], ps_ao[:], gate2[:], op=MULT)
            nc.vector.tensor_tensor(attn_out[:], attn_out[:], a_own_sb[:], op=ADD)

            # --- adaLN2
            inv2, negmi2 = ln_stats(wk3, attn_out[:], CA, "ln2")
            a2_ln = wk3.tile([P, CA], BF16, tag="a2_ln", name="a2_ln", bufs=1)
            nc.scalar.activation(a2_ln[:], attn_out[:], AF.Identity,
                                 bias=negmi2[:], scale=inv2[:])
            a2_lnT = wk3.tile([P, CAT, NQ], BF16, tag="a2_lnT", name="a2_lnT", bufs=1)
            transposes([(a2_ln[:, j * P:(j + 1) * P], a2_lnT[:, j, :])
                        for j in range(CAT)], "a2")
            tT = wk3.tile([P, CAT, NQ], BF16, tag="tT", name="tT", bufs=1)
            for j in range(CAT):
                pg = psB.tile([P, NQ], F32, tag="psB", name=f"pg2{j}")
                pb = psB.tile([P, NQ], F32, tag="psB", name=f"pb2{j}")
                for i in range(CST):
                    nc.tensor.matmul(pg[:], W_sg2[:, i, j * P:(j + 1) * P],
                                     s_own_lnT[:, i, :],
                                     start=(i == 0), stop=(i == CST - 1))
                for i in range(CST):
                    nc.tensor.matmul(pb[:], W_sb2[:, i, j * P:(j + 1) * P],
                                     s_own_lnT[:, i, :],
                                     start=(i == 0), stop=(i == CST - 1))
                sig = wk3.tile([P, NQ], BF16, tag="sg2s", name=f"sg2s{j}")
                nc.scalar.activation(sig[:], pg[:], AF.Sigmoid, bias=bb2[:, j:j + 1])
                sbv = wk3.tile([P, NQ], BF16, tag="sb2s", name=f"sb2s{j}")
                nc.scalar.activation(sbv[:], pb[:], AF.Identity,
                                     bias=bb2[:, 6 + j:7 + j])
                nc.vector.tensor_tensor(sig[:], sig[:], a2_lnT[:, j, :], op=MULT)
                nc.vector.tensor_tensor(tT[:, j, :], sig[:], sbv[:], op=ADD)

            # --- transition: bbT = silu(t@W1) * (t@W2)  (feature layout)
            bbT = wk3.tile([P, F1T, NQ], BF16, tag="bbT", name="bbT", bufs=1)
            for j in range(F1T):
                p1 = psB.tile([P, NQ], F32, tag="psB", name=f"p1_{j}")
                p2 = psB.tile([P, NQ], F32, tag="psB", name=f"p2_{j}")
                for i in range(CAT):
                    nc.tensor.matmul(p1[:], W_1[:, i, j * P:(j + 1) * P], tT[:, i, :],
                                     start=(i == 0), stop=(i == CAT - 1))
                for i in range(CAT):
                    nc.tensor.matmul(p2[:], W_2[:, i, j * P:(j + 1) * P], tT[:, i, :],
                                     start=(i == 0), stop=(i == CAT - 1))
                sil = wk3.tile([P, NQ], BF16, tag="sil", name=f"sil{j}")
                nc.scalar.activation(sil[:], p1[:], AF.Silu)
                nc.vector.tensor_tensor(bbT[:, j, :], sil[:], p2[:], op=MULT)

            ps_ff = token_mm([bbT[:, j, :] for j in range(F1T)], W_b, None, "ps_ff")
            ps_g3 = token_mm([sT_own[:, j, :] for j in range(CST)], W_s, 2, "ps_g3")
            gate3 = wk3.tile([P, CA], BF16, tag="gate3", name="gate3", bufs=1)
            nc.scalar.activation(gate3[:], ps_g3[:], AF.Sigmoid)
            out_f = wk3.tile([P, CA], F32, tag="out_f", name="out_f", bufs=1)
            nc.vector.tensor_tensor(out_f[:], ps_ff[:], gate3[:], op=MULT)
            nc.vector.tensor_tensor(out_f[:], out_f[:], attn_out[:], op=ADD)
            nc.sync.dma_start(out_own[:], out_f[:])
            if taps:
                nc.sync.dma_start(taps["t_otok"][:], o_tok[:])
                nc.sync.dma_start(taps["t_r"][:], r_sb[:])
                nc.sync.dma_start(taps["t_attn"][:], attn_out[:])
                nc.sync.dma_start(taps["t_gate2"][:], gate2[:])
                nc.sync.dma_start(taps["t_bias"][:], bias_all[:].rearrange("p a b c -> p (a b c)"))
                nc.sync.dma_start(taps["t_qpad"][:], q_pad[:].rearrange("p a b -> p (a b)"))
                nc.sync.dma_start(taps["t_kpad"][:], k_pad[:].rearrange("p a b -> p (a b)"))
                nc.sync.dma_start(taps["t_vtok"][:], v_tok[:].rearrange("p a b -> p (a b)"))
                nc.sync.dma_start(taps["t_xown"][:], x_own[:].rearrange("p a b -> p (a b)"))
                nc.sync.dma_start(taps["t_gateown"][:], gate_own[:])

    nc.compile()
    return nc


def _prep_host(inputs):
    """Fold weights on host; returns per-core in_maps."""
    f32 = lambda x: np.asarray(x, dtype=np.float32)
    a = f32(inputs["a"]).reshape(N, CA)
    s = f32(inputs["s"]).reshape(N, CS)
    z = np.asarray(inputs["z"]).reshape(N, N, CZ)

    g1, b1 = f32(inputs["adaln1_sln_g"]), f32(inputs["adaln1_sln_b"])
    sgW1 = g1[:, None] * f32(inputs["adaln1_sg_W"])
    bsg1 = f32(inputs["adaln1_sg_b"]) + b1 @ f32(inputs["adaln1_sg_W"])
    sbW1 = g1[:, None] * f32(inputs["adaln1_sb_W"])
    bsb1 = b1 @ f32(inputs["adaln1_sb_W"])

    sc = D ** -0.5
    Wq = f32(inputs["Wq"]) * sc
    bq = f32(inputs["bq"]) * sc
    Wk = f32(inputs["Wk"])
    Wq_pad = np.zeros((CA, HP * P), np.float32)
    Wk_pad = np.zeros((CA, HP * P), np.float32)
    bq_pad = np.zeros(HP * P, np.float32)
    for h in range(H):
        Wq_pad[:, 64 * h:64 * h + D] = Wq[:, D * h:D * (h + 1)]
        Wk_pad[:, 64 * h:64 * h + D] = Wk[:, D * h:D * (h + 1)]
        bq_pad[64 * h:64 * h + D] = bq[D * h:D * (h + 1)]

    zg = f32(inputs["zln_g"])
    Wzp = zg[:, None] * f32(inputs["Wz"])
    Wzpp = Wzp - np.ones((CZ, 1), np.float32) * (Wzp.sum(0)[None, :] / CZ)
    Wz_ext = np.concatenate(
        [Wzpp, np.full((CZ, 1), 1.0 / CZ, np.float32),
         np.full((CZ, 1), 1.0 / CZ, np.float32)], axis=1)

    g2, b2 = f32(inputs["adaln2_sln_g"]), f32(inputs["adaln2_sln_b"])
    sgW2 = g2[:, None] * f32(inputs["adaln2_sg_W"])
    bsg2 = f32(inputs["adaln2_sg_b"]) + b2 @ f32(inputs["adaln2_sg_W"])
    sbW2 = g2[:, None] * f32(inputs["adaln2_sb_W"])
    bsb2 = b2 @ f32(inputs["adaln2_sb_W"])

    bias_blob_v = np.zeros((P, 20), np.float32)
    for j in range(CAT):
        bias_blob_v[:, j] = bsg1[j * P:(j + 1) * P]
        bias_blob_v[:, 6 + j] = bsb1[j * P:(j + 1) * P]
    for j in range(HP):
        bias_blob_v[:, 12 + j] = bq_pad[j * P:(j + 1) * P]
    bias_blob2_v = np.zeros((P, 12), np.float32)
    for j in range(CAT):
        bias_blob2_v[:, j] = bsg2[j * P:(j + 1) * P]
        bias_blob2_v[:, 6 + j] = bsb2[j * P:(j + 1) * P]
    bias_rows_v = np.concatenate(
        [f32(inputs["blast"]), f32(inputs["bo"]), f32(inputs["bs"])]
    ).reshape(1, 3 * CA)

    B = lambda x: np.ascontiguousarray(np.asarray(x, dtype=np.float32)).astype(bf16)
    shared = dict(
        a_full=np.ascontiguousarray(a), s_full=np.ascontiguousarray(s),
        w_sg1=B(sgW1), w_sb1=B(sbW1), w_q=B(Wq_pad), w_k=B(Wk_pad),
        w_v=B(inputs["Wv"]), w_g=B(inputs["Wg"]),
        w_o=B(inputs["Wo"]), w_sg2=B(sgW2), w_sb2=B(sbW2),
        w_1=B(inputs["W1"]), w_2=B(inputs["W2"]),
        w_b=B(inputs["Wb"]), w_s=B(inputs["Ws"]),
        w_last=B(inputs["Wlast"]), w_z=B(Wz_ext),
        bias_blob=bias_blob_v, bias_blob2=bias_blob2_v, bias_rows=B(bias_rows_v),
        ident_in=np.eye(P, dtype=np.float32).astype(bf16),
        ones_row_in=np.ones((1, P), bf16), ones_col_in=np.ones((P, 1), bf16),
    )
    z_bf = np.asarray(z, dtype=np.float32).astype(bf16)
    in_maps = []
    for c in range(NCORES):
        m = dict(shared)
        m["a_own"] = np.ascontiguousarray(a[c * NQ:(c + 1) * NQ])
        m["s_own"] = np.ascontiguousarray(s[c * NQ:(c + 1) * NQ])
        m["z_sh"] = np.ascontiguousarray(z_bf[c * NQ:(c + 1) * NQ]).reshape(NQ * N, CZ)
        in_maps.append(m)
    return in_maps


def kernel(**inputs):
    if "nc" not in _CACHE:
        _CACHE["nc"] = build_kernel()
    nc = _CACHE["nc"]
    in_maps = _prep_host(inputs)
    res = bass_utils.run_bass_kernel_spmd(nc, in_maps, core_ids=list(range(NCORES)))
    out = np.concatenate([res.results[c]["out_own"] for c in range(NCORES)], axis=0)
    out_a = out.reshape(1, N, CA)
    return (out_a, np.asarray(inputs["s"]), np.asarray(inputs["z"]))
